# revision 28
# baseline (speedup 1.0000x reference)
"""AFT2D attention Trainium2 kernel (8 NeuronCores, data-parallel over batch).

Math: the reference's 5x5 windowed attention with positional bias
    wgt = exp(w_h[ii]*(di-h) + w_v[jj]*(dj-w) + k[h+di, w+dj]) * mask
factorizes exactly: exp(bias) separates into per-row and per-column factors,
so with ek = exp(k), u = ek*v, s = sum_d ek:
    out  = A @ (B ∘w u)      (two banded 64x64 matrix passes, h then w)
    norm = A @ (B ∘w s)
    y    = (out @ Wp^T) / (norm + eps)      (normalize commutes past Wp)
where A[h,h'] = exp(w_h[h'-h+R]*((h'-h)-h)) on the band, B likewise over w.

Per-core pipeline (b_loc=2 images; partitions = (b,w), then (h,b) after the
h<->w shuffle):
  1. k|v matmul: x-tiles (pre-transposed on host, bf16) as PE stationary,
     streaming [Wk^T | Wv^T]; psum [128=(b,w), 512] per h.
  2. ACT exp (accum_out -> s), DVE u = ek * v -> u slab [(b,w), (h,d)] bf16.
  3. Horizontal pass: matmul per 512-chunk, stationary blkdiag(B^T,B^T);
     drains -> T slab.
  4. h<->w shuffle: ONE dma per h writes Tt rows (2h+0, 2h+1) -- the
     (h,b)-interleaved row order makes source/dest iteration orders match,
     so 64 DMAs move the whole 4MB.
  5. Vertical pass, swapped operands (lhsT = Tt chunk, rhs = row-permuted
     kron of A^T) -> psum comes out feature-major [d-half, (b,h)] = exactly
     the projection's lhsT; no transpose anywhere.
  6. Projection matmul + drain scaled by 1/norm (per-partition scale), bf16.
Norm path runs in f32 on the side (tiny matmuls; per-b col-tiled output).
"""
import sys

sys.path.insert(0, "/opt/trn_rl_repo")

import numpy as np
import ml_dtypes

import concourse.bass as bass
import concourse.mybir as mybir
import concourse.tile as tile
from concourse.bass_utils import run_bass_kernel_spmd

bf16 = ml_dtypes.bfloat16

N_CORES = 8
B_FULL, H, W, C = 16, 64, 64, 256
D = 256   # HID
O = 256   # OUT
R = 2
B_LOC = B_FULL // N_CORES  # 2

LAST_RESULT = None
_CACHED_NC = None


def _split_multi_waits(nc, max_waits=1):
    """This container's walrus accepts at most ONE sync-wait per instruction;
    hoist extras into standalone same-engine no-ops (order-preserving)."""
    n_new = 0
    for func in nc.m.functions:
        for blk in func.blocks:
            new_insts = []
            for inst in blk.instructions:
                si = inst.sync_info
                if si is not None and len(si.on_wait) > max_waits:
                    waits = list(si.on_wait)
                    for w in waits[:-max_waits]:
                        nop = mybir.InstNoOp(
                            name=f"waitsplit-{n_new}-{inst.name}", ins=[], outs=[])
                        nop.engine = inst.engine
                        nop.sync_info = mybir.SyncInfo(on_wait=[w], on_update=[])
                        new_insts.append(nop)
                        n_new += 1
                    si.on_wait = waits[-max_waits:]
                new_insts.append(inst)
            blk.instructions = new_insts
    return n_new


# packed bf16 constant blob column offsets
_WKV_OFF = 0                 # [128, 2, 512]
_WP_OFF = _WKV_OFF + 1024    # [128, 2, 256]
_BH_OFF = _WP_OFF + 512      # [128, 128]
_AV_OFF = _BH_OFF + 128      # [128, 128]
_CB_COLS = _AV_OFF + 128
# f32 blob: bh_f32 [128,128] then a64t [64,64] (cols 128..192)
_FB_COLS = 192


def _build_nc():
    fp32 = mybir.dt.float32
    bft = mybir.dt.bfloat16

    nc = bass.Bass()
    xt_ext = nc.declare_dram_parameter("xt", [128, H, 2, 128], bft, isOutput=False)
    cb_ext = nc.declare_dram_parameter("cblob", [128, _CB_COLS], bft, isOutput=False)
    fb_ext = nc.declare_dram_parameter("fblob", [128, _FB_COLS], fp32, isOutput=False)
    y_ext = nc.declare_dram_parameter("y", [B_LOC, H, W, O], bft, isOutput=True)
    # HBM bounce for the h<->w shuffle: layout (h, b, w, d) -- strided write,
    # contiguous read (the read is on the critical path after the barrier)
    tbounce = nc.dram_tensor("tbounce", [H, B_LOC, W, D], bft)

    with tile.TileContext(nc) as tc:
        with (
            tc.tile_pool(name="const", bufs=1) as cpool,
            tc.tile_pool(name="ek", bufs=6) as ek_pool,
            tc.tile_pool(name="slab", bufs=1) as slab_pool,
            tc.tile_pool(name="ot", bufs=4) as ot_pool,
            tc.tile_pool(name="y", bufs=4) as y_pool,
        ):
            # ---- constants; wkv first (first matmul's dependency)
            cblob = cpool.tile([128, _CB_COLS], bft)
            nc.sync.dma_start(out=cblob[:, 0:_WP_OFF], in_=cb_ext[:, 0:_WP_OFF])
            nc.sync.dma_start(out=cblob[:, _WP_OFF:], in_=cb_ext[:, _WP_OFF:])
            fblob = cpool.tile([128, _FB_COLS], fp32)
            nc.sync.dma_start(out=fblob[:], in_=fb_ext[:])
            wkv_sb = cblob[:, _WKV_OFF:_WKV_OFF + 1024].rearrange(
                "p (c d) -> p c d", c=2)
            wp_sb = cblob[:, _WP_OFF:_WP_OFF + 512].rearrange(
                "p (c d) -> p c d", c=2)
            bh_bd = cblob[:, _BH_OFF:_BH_OFF + 128]
            av_mix = cblob[:, _AV_OFF:_AV_OFF + 128]
            bh_f32 = fblob[:, 0:128]
            a64t = fblob[0:64, 128:192]

            # ---- x in (16 pieces so the first matmuls start early)
            xt_slab = slab_pool.tile([128, H, 2, 128], bft)
            for q in range(16):
                nc.sync.dma_start(out=xt_slab[:, q * 4:(q + 1) * 4],
                                  in_=xt_ext[:, q * 4:(q + 1) * 4])

            u_slab = slab_pool.tile([128, H, D], bft)      # [(b,w), h, d]
            t_slab = slab_pool.tile([128, H, D], bft)      # [(b,w), h, d]
            tt_slab = slab_pool.tile([128, W, D], bft)     # [(h,b)=2h+b, w, d]
            s_slab = slab_pool.tile([128, H], fp32)        # [(b,w), h]
            u_flat = u_slab.rearrange("p h d -> p (h d)")
            t_flat = t_slab.rearrange("p h d -> p (h d)")
            tt_flat = tt_slab.rearrange("p w d -> p (w d)")

            tb_w = tbounce.rearrange("h b w d -> b w h d")   # leg-1 write view
            tb_r = tbounce.rearrange("h b w d -> (h b) w d")  # leg-2 read view
            sth = cpool.tile([64, 128], fp32, tag="sth")

            with (
                tc.tile_pool(name="ps_kv", bufs=4, space="PSUM") as ps_kv_pool,
                tc.tile_pool(name="ps_t", bufs=4, space="PSUM") as ps_t_pool,
            ):
                # ---- phase 1: k|v matmuls + exp + u
                # ---- phase 3 (interleaved): horizontal pass, contracts w'
                # ---- phase 4 (interleaved): HBM-bounce shuffle, quartered
                n_hchunks = H * D // 512      # 32; chunk c covers h = 2c, 2c+1
                for h in range(H):
                    ps_kv = ps_kv_pool.tile([128, 2 * D], fp32)
                    nc.tensor.matmul(ps_kv[:], xt_slab[:, h, 0, :], wkv_sb[:, 0, :],
                                     start=True, stop=False)
                    nc.tensor.matmul(ps_kv[:], xt_slab[:, h, 1, :], wkv_sb[:, 1, :],
                                     start=False, stop=True)
                    ek = ek_pool.tile([128, D], bft)
                    nc.scalar.activation(ek[:], ps_kv[:, 0:D],
                                         mybir.ActivationFunctionType.Exp,
                                         accum_out=s_slab[:, h:h + 1])
                    nc.vector.tensor_mul(u_slab[:, h, :], ek[:], ps_kv[:, D:2 * D])

                    if h % 8 == 7:
                        # horizontal matmuls batched x4: one stationary load
                        # of bh_bd per run instead of per matmul
                        for c in range(4 * (h // 8), 4 * (h // 8) + 4):
                            ps_t = ps_t_pool.tile([128, 512], fp32)
                            nc.tensor.matmul(ps_t[:], bh_bd[:],
                                             u_flat[:, c * 512:(c + 1) * 512],
                                             start=True, stop=True)
                            # drains: 1 of 8 on ACT, 7 of 8 on DVE
                            if c % 8 == 0:
                                nc.scalar.copy(
                                    t_flat[:, c * 512:(c + 1) * 512], ps_t[:])
                            else:
                                nc.vector.tensor_copy(
                                    t_flat[:, c * 512:(c + 1) * 512], ps_t[:])
                        # bounce leg 1 for this h-range: strided HBM write
                        q = h // 8
                        nc.sync.dma_start(
                            out=tb_w[:, :, q * 8:(q + 1) * 8, :],
                            in_=t_slab[:, q * 8:(q + 1) * 8, :])

                # bounce leg 2: contiguous HBM reads, split by w-range so the
                # vertical pass streams per-piece instead of one barrier
                for qw in range(8):
                    nc.sync.dma_start(
                        out=tt_slab[:, qw * 8:(qw + 1) * 8, :],
                        in_=tb_r[:, qw * 8:(qw + 1) * 8, :])

                # norm-horizontal (swapped: out comes transposed [h, (b,w)])
                ps_sh = ps_t_pool.tile([64, 128], fp32, tag="ps_t")
                nc.tensor.matmul(ps_sh[:], s_slab[:], bh_f32[:],
                                 start=True, stop=True)
                nc.vector.tensor_copy(sth[:], ps_sh[:])

            with (
                tc.tile_pool(name="ps_g", bufs=4, space="PSUM") as ps_g_pool,
                tc.tile_pool(name="ps_y", bufs=4, space="PSUM") as ps_y_pool,
            ):
                # norm-vertical: per-b matmuls into partition halves (col tiling)
                ps_n = ps_y_pool.tile([128, 64], fp32, tag="ps_y")
                for b in range(B_LOC):
                    nc.tensor.matmul(ps_n[b * 64:(b + 1) * 64, :], a64t[:],
                                     sth[:, b * 64:(b + 1) * 64],
                                     start=True, stop=True,
                                     tile_position=(0, b * 64))
                ntmp = cpool.tile([128, 64], fp32, tag="ntmp")
                nc.vector.tensor_scalar_add(ntmp[:], ps_n[:], 1e-8)
                rnorm = cpool.tile([128, 64], fp32, tag="rnorm")
                nc.vector.reciprocal(rnorm[:], ntmp[:])

                # ---- phase 5+6: vertical pass (swapped -> feature-major) + proj
                for g in range(W // 2):           # w-pair per psum group
                    ps_g = ps_g_pool.tile([128, 512], fp32)
                    for q in range(4):
                        cch = g * 4 + q   # chunk = (w = cch>>1, dhalf = cch&1)
                        nc.tensor.matmul(ps_g[:, q * 128:(q + 1) * 128],
                                         tt_flat[:, cch * 128:(cch + 1) * 128],
                                         av_mix[:], start=True, stop=True)
                    ot = ot_pool.tile([128, 512], bft)
                    nc.scalar.copy(ot[:], ps_g[:])

                    for wi in range(2):
                        w = 2 * g + wi
                        base = wi * 256
                        ps_y = ps_y_pool.tile([128, O], fp32)
                        nc.tensor.matmul(ps_y[:], ot[:, base:base + 128],
                                         wp_sb[:, 0, :], start=True, stop=False)
                        nc.tensor.matmul(ps_y[:], ot[:, base + 128:base + 256],
                                         wp_sb[:, 1, :], start=False, stop=True)
                        if w % 4 == 0:
                            y4 = y_pool.tile([128, 4, O], bft)
                        yt = y4[:, w % 4, :]
                        # y drains: 1 of 8 on ACT, 7 of 8 on DVE
                        if w % 8 == 0:
                            nc.scalar.mul(yt[:], ps_y[:], rnorm[:, w:w + 1])
                        else:
                            nc.vector.tensor_scalar_mul(yt[:], ps_y[:],
                                                        rnorm[:, w:w + 1])
                        if w % 4 == 3:
                            w0 = w - 3
                            nc.sync.dma_start(
                                out=y_ext[:, :, w0:w0 + 4, :].rearrange(
                                    "b h w o -> (b h) w o"),
                                in_=y4[:])

    _split_multi_waits(nc)
    return nc


def _host_prep(x, w_h, w_v, Wk, Wv, Wp):
    """Build per-core input maps (all layout/packing on host, compute on device)."""
    A = np.zeros((H, H), np.float32)
    Bm = np.zeros((W, W), np.float32)
    for h in range(H):
        for hp in range(max(0, h - R), min(H, h + R + 1)):
            A[h, hp] = np.exp(w_h[hp - h + R] * ((hp - h) - h))
    for w in range(W):
        for wp in range(max(0, w - R), min(W, w + R + 1)):
            Bm[w, wp] = np.exp(w_v[wp - w + R] * ((wp - w) - w))

    eye2 = np.eye(2, dtype=np.float32)
    bh_bd = np.kron(eye2, Bm.T)                      # lhsT for horizontal
    # vertical rhs, rows (h',b)-interleaved, cols (b,h) b-major:
    av_mix = np.zeros((128, 128), np.float32)
    for b in range(B_LOC):
        for h in range(H):
            for hp in range(max(0, h - R), min(H, h + R + 1)):
                av_mix[2 * hp + b, 64 * b + h] = A[h, hp]

    # wkv[ci, cc, j] = Wk[j, cc*128+ci] (j<256) else Wv[j-256, ...]
    wkv = np.concatenate([Wk.T, Wv.T], axis=1)       # [C, 2D]
    wkv = wkv.reshape(2, 128, 2 * D).transpose(1, 0, 2)      # [ci, cc, 512]
    wp = Wp.T.reshape(2, 128, O).transpose(1, 0, 2)          # [di, dc, o]

    cblob = np.empty((128, _CB_COLS), np.float32)
    cblob[:, _WKV_OFF:_WKV_OFF + 1024] = wkv.reshape(128, 1024)
    cblob[:, _WP_OFF:_WP_OFF + 512] = wp.reshape(128, 512)
    cblob[:, _BH_OFF:_BH_OFF + 128] = bh_bd
    cblob[:, _AV_OFF:_AV_OFF + 128] = av_mix
    cblob = cblob.astype(bf16)

    fblob = np.zeros((128, _FB_COLS), np.float32)
    fblob[:, 0:128] = bh_bd
    fblob[0:64, 128:192] = A.T

    in_maps = []
    for c in range(N_CORES):
        xl = x[c * B_LOC:(c + 1) * B_LOC]            # (2, 64, 64, 256)
        t = xl.reshape(B_LOC, H, W, 2, 128)          # b h w cc ci
        xt = np.ascontiguousarray(
            t.transpose(4, 1, 3, 0, 2).reshape(128, H, 2, 128)).astype(bf16)
        in_maps.append({"xt": xt, "cblob": cblob, "fblob": fblob})
    return in_maps


def kernel(x, w_h, w_v, Wk, Wv, Wp):
    global LAST_RESULT, _CACHED_NC
    x = np.asarray(x, dtype=np.float32)
    w_h = np.asarray(w_h, dtype=np.float32)
    w_v = np.asarray(w_v, dtype=np.float32)
    Wk = np.asarray(Wk, dtype=np.float32)
    Wv = np.asarray(Wv, dtype=np.float32)
    Wp = np.asarray(Wp, dtype=np.float32)

    in_maps = _host_prep(x, w_h, w_v, Wk, Wv, Wp)
    if _CACHED_NC is None:
        _CACHED_NC = _build_nc()
    res = run_bass_kernel_spmd(_CACHED_NC, in_maps, core_ids=list(range(N_CORES)))
    LAST_RESULT = res

    out = np.empty((B_FULL, H, W, O), np.float32)
    for c in range(N_CORES):
        yc = np.asarray(res.results[c]["y"]).astype(np.float32)
        out[c * B_LOC:(c + 1) * B_LOC] = yc
    return out
